# revision 15
# baseline (speedup 1.0000x reference)
"""GCN-BMP encoder (2x RelationalGraphConv + highway) on 8 Trainium2 NeuronCores.

Strategy (sharding_hint: shard nodes + incident edges, replicate weights):
  - Nodes sharded in contiguous ranges of 12500 across 8 cores; edges assigned
    to the core owning their dst node.
  - Per core, edges are grouped by (relation r, src-chunk s) -- src chunks of
    25000 nodes keep dma_gather's int16 index range -- and each group is split
    into "rounds" (k-th edge per destination) so each dma_scatter_add call has
    unique target rows (concurrent CCE adds to the same row race on HW).
  - Scatter accumulates raw sums into per-relation upd tensors; the 1/count
    mean scaling is applied afterwards with a host-precomputed inverse-count
    matrix (counts depend only on the int edge list, not on x).
  - Dense part: upd tiles are transposed on TensorE, then out.T = W.T @ upd.T
    (+ self-loop term) so the highway layers run entirely in feature-major
    layout without further transposes.
  - g1 is exchanged between layers with an AllGather collective (each core
    contributes its 12500x64 shard); layer 2 gathers from the shared copy.
"""
import sys

if "/opt/trn_rl_repo" not in sys.path:
    sys.path.insert(0, "/opt/trn_rl_repo")

import numpy as np

N = 100000
E = 1200000
D = 64
R = 4
C = 8

FULL_CFG = dict(
    N=N, E=E, C=C,
    NPC=N // C,          # nodes per core (dst shard)
    NPAD=13184,          # 103*128; >= NPC + max scatter pad per round + 128
    NSC=4, SCN=25000,    # src chunks for gather tables (int16 index limit)
    MCH=512,             # dense-phase column chunk
)


def _wrap_idx(idx: np.ndarray, cols: int) -> np.ndarray:
    """[M] ints -> [128, cols] int16: i at [i%16, i//16], replicated x8."""
    m = idx.shape[0]
    assert m % 16 == 0 and m // 16 == cols
    a = idx.astype(np.int16).reshape(cols, 16).T
    return np.tile(a, (8, 1))


def _prep(cfg, src, dst, rel):
    """Host-side graph partitioning. Returns (layout, per-core arrays).

    layout: rounds[r][s] = list of round sizes (multiples of 128, shared by
    all cores); col_off[(r,s,k)] = column offset into the idx blobs.
    per-core: gidx_blob/sidx_blob [128, GCOLS] int16.
    """
    NPC, NPAD, NSC, SCN = cfg["NPC"], cfg["NPAD"], cfg["NSC"], cfg["SCN"]
    nC = cfg["C"]
    src = np.asarray(src).astype(np.int64)
    dst = np.asarray(dst).astype(np.int64)
    rel = np.asarray(rel).astype(np.int64)

    owner = dst // NPC
    schunk = src // SCN
    # per core / r / s: (gather_local_src, scatter_local_dst, rank_within_dst)
    percore = []
    for c in range(nC):
        groups = {}
        mc = owner == c
        for r in range(R):
            for s in range(NSC):
                sel = mc & (rel == r) & (schunk == s)
                dl = dst[sel] - c * NPC
                sl = src[sel] - s * SCN
                order = np.argsort(dl, kind="stable")
                dl, sl = dl[order], sl[order]
                # rank of each edge within its dst run (dst-sorted)
                if dl.size:
                    first = np.searchsorted(dl, dl)
                    rank = np.arange(dl.size) - first
                else:
                    rank = dl
                groups[(r, s)] = (sl, dl, rank)
        percore.append(groups)

    rounds = {}
    for r in range(R):
        for s in range(NSC):
            nk = max(
                (int(percore[c][(r, s)][2].max()) + 1 if percore[c][(r, s)][2].size else 0)
                for c in range(nC)
            )
            sizes = []
            for k in range(nk):
                mx = max(
                    int((percore[c][(r, s)][2] == k).sum()) for c in range(nC)
                )
                sizes.append(-(-mx // 128) * 128)
            rounds[(r, s)] = sizes

    # Split every round into sub-calls of <= SUBMAX tokens (HW descriptor-ring
    # limit per SWDGE call) and assign each to one of P accumulator tensors.
    SUBMAX = 1024
    P = 2
    col_off = {}
    calls = {}  # (r, s) -> list of (col_off, size, parity)
    off = 0
    pcounter = 0
    for r in range(R):
        for s in range(NSC):
            cl = []
            for k, sz in enumerate(rounds[(r, s)]):
                col_off[(r, s, k)] = off
                sub = 0
                while sub < sz:
                    ssz = min(SUBMAX, sz - sub)
                    cl.append((off + sub // 16, ssz, pcounter % P))
                    pcounter += 1
                    sub += ssz
                off += sz // 16
            calls[(r, s)] = cl
    gcols = off

    gblobs, sblobs = [], []
    max_pad = 0
    for c in range(nC):
        gb = np.zeros((128, gcols), np.int16)
        sb = np.zeros((128, gcols), np.int16)
        for r in range(R):
            for s in range(NSC):
                sl, dl, rank = percore[c][(r, s)]
                for k, sz in enumerate(rounds[(r, s)]):
                    m = rank == k
                    g = sl[m]
                    d = dl[m]
                    pad = sz - g.size
                    max_pad = max(max_pad, pad)
                    gfull = np.concatenate([g, np.zeros(pad, np.int64)])
                    sfull = np.concatenate([d, NPC + np.arange(pad)])
                    o = col_off[(r, s, k)]
                    gb[:, o : o + sz // 16] = _wrap_idx(gfull, sz // 16)
                    sb[:, o : o + sz // 16] = _wrap_idx(sfull, sz // 16)
        gblobs.append(gb)
        sblobs.append(sb)
    assert max_pad <= NPAD - NPC, (max_pad, NPAD, NPC)

    layout = dict(rounds=rounds, col_off=col_off, gcols=gcols, calls=calls, P=P)
    return layout, gblobs, sblobs


def _build(cfg, layout):
    import concourse.bacc as bacc
    import concourse.tile as tile
    from concourse import mybir

    f32 = mybir.dt.float32
    i16 = mybir.dt.int16
    AF = mybir.ActivationFunctionType

    NPC, NPAD, NSC, SCN, MCH = (
        cfg["NPC"], cfg["NPAD"], cfg["NSC"], cfg["SCN"], cfg["MCH"]
    )
    nN, nC = cfg["N"], cfg["C"]
    calls, gcols, P = layout["calls"], layout["gcols"], layout["P"]
    max_call = max((sz for cl in calls.values() for (_, sz, _) in cl), default=128)

    nc = bacc.Bacc("TRN2", target_bir_lowering=False, debug=False, num_devices=nC)

    x_d = nc.dram_tensor("x", [nN, D], f32, kind="ExternalInput")
    xT_d = nc.dram_tensor("xT", [D, NPAD], f32, kind="ExternalInput")
    gidx_d = nc.dram_tensor("gidx", [128, gcols], i16, kind="ExternalInput")
    sidx_d = nc.dram_tensor("sidx", [128, gcols], i16, kind="ExternalInput")
    inv_d = nc.dram_tensor("inv", [NPAD, R * D], f32, kind="ExternalInput")
    eye_d = nc.dram_tensor("eye", [128, 128], f32, kind="ExternalInput")
    wdecl = {}
    for L in (1, 2):
        for nm, shp in [
            ("wt", [2 * D, 2 * D]), ("wst", [D, D]), ("cb", [D, 1]),
            ("pwth", [D, D]), ("pwtp", [D, D]), ("pb", [D, 1]),
            ("twth", [D, D]), ("twtp", [D, D]), ("tb", [D, 1]),
        ]:
            wdecl[f"{nm}{L}"] = nc.dram_tensor(f"{nm}{L}", shp, f32, kind="ExternalInput")
    out_d = nc.dram_tensor("out", [NPC, D], f32, kind="ExternalOutput")

    upds = {(L, r, p): nc.dram_tensor(f"upd{L}_{r}_{p}", [NPAD, D], f32)
            for L in (1, 2) for r in range(R) for p in range(P)}
    g1rows_d = nc.dram_tensor("g1rows", [NPC, D], f32)
    g1full_d = nc.dram_tensor("g1full", [nN, D], f32, addr_space="Shared")
    h1T_d = nc.dram_tensor("h1T", [D, NPAD], f32)
    g1T_d = nc.dram_tensor("g1T", [D, NPAD], f32)

    with tile.TileContext(nc) as tc:
        with (
            tc.tile_pool(name="const", bufs=1) as constp,
            tc.tile_pool(name="zero", bufs=1) as zerop,
            tc.tile_pool(name="idx", bufs=4) as idxp,
            tc.tile_pool(name="gat", bufs=3) as gatp,
            tc.tile_pool(name="su", bufs=3) as sup,
            tc.tile_pool(name="cw", bufs=2) as cwp,
            tc.tile_pool(name="row", bufs=3) as rowp,
            tc.tile_pool(name="pt", bufs=2, space="PSUM") as ptp,
            tc.tile_pool(name="ph", bufs=1, space="PSUM") as php,
            tc.tile_pool(name="ppr", bufs=1, space="PSUM") as pprp,
            tc.tile_pool(name="pg", bufs=1, space="PSUM") as pgp,
            tc.tile_pool(name="po", bufs=2, space="PSUM") as pop,
        ):
            # ---- constants ----
            eye = constp.tile([128, 128], f32, tag="eye")
            nc.sync.dma_start(eye[:], eye_d[:])
            W = {}
            for L in (1, 2):
                for nm in ["wt", "wst", "cb", "pwth", "pwtp", "pb", "twth", "twtp", "tb"]:
                    t = constp.tile(list(wdecl[f"{nm}{L}"].shape), f32, tag=f"{nm}{L}")
                    nc.sync.dma_start(t[:], wdecl[f"{nm}{L}"].ap())
                    W[(nm, L)] = t
            # ---- zero the upd accumulators ----
            zcols = NPAD * D // 128 // 2
            z = zerop.tile([128, zcols], f32)
            nc.vector.memset(z[:], 0.0)
            for key, t in upds.items():
                flat = t.ap().rearrange("(a b) k -> a (b k)", a=128)
                nc.sync.dma_start(flat[:, :zcols], z[:])
                nc.sync.dma_start(flat[:, zcols:], z[:])

            def edge_phase(L, table):
                emitted = 0
                for r in range(R):
                    for s in range(NSC):
                        for (o, sz, p) in calls[(r, s)]:
                            if emitted >= cfg.get("MAXROUNDS", 1 << 30):
                                continue
                            emitted += 1
                            gi = idxp.tile([128, max_call // 16], i16, tag="gi")
                            si = idxp.tile([128, max_call // 16], i16, tag="si")
                            nc.sync.dma_start(
                                gi[:, : sz // 16], gidx_d[:, o : o + sz // 16]
                            )
                            nc.sync.dma_start(
                                si[:, : sz // 16], sidx_d[:, o : o + sz // 16]
                            )
                            G = gatp.tile([128, (max_call // 128) * D], f32, tag="G")
                            g3 = G[:, : (sz // 128) * D].rearrange(
                                "p (t e) -> p t e", e=D
                            )
                            nc.gpsimd.dma_gather(
                                g3, table[s * SCN : (s + 1) * SCN, :],
                                gi[:, : sz // 16], sz, sz, D,
                            )
                            nc.gpsimd.dma_scatter_add(
                                upds[(L, r, p)].ap(), g3, si[:, : sz // 16], sz, sz, D,
                            )

            def dense_phase(L, rhs2T_d, prevT_d, hT_out_d, oT_out_d, rows_dst):
                wt, wst, cb = W[("wt", L)], W[("wst", L)], W[("cb", L)]
                pwth, pwtp, pb = W[("pwth", L)], W[("pwtp", L)], W[("pb", L)]
                twth, twtp, tb = W[("twth", L)], W[("twtp", L)], W[("tb", L)]
                same_rp = rhs2T_d is prevT_d
                for ch0 in range(0, NPAD, MCH):
                    Wc = min(MCH, NPAD - ch0)
                    t0 = cwp.tile([128, MCH], f32, tag="t0")
                    t1 = cwp.tile([128, MCH], f32, tag="t1")
                    for j in range(Wc // 128):
                        m0 = ch0 + j * 128
                        su = sup.tile([128, R * D], f32, tag="su")
                        ub = sup.tile([128, R * D], f32, tag="ub")
                        for r in range(R):
                            nc.sync.dma_start(
                                su[:, r * D : (r + 1) * D],
                                upds[(L, r, 0)][m0 : m0 + 128, :],
                            )
                            nc.sync.dma_start(
                                ub[:, r * D : (r + 1) * D],
                                upds[(L, r, 1)][m0 : m0 + 128, :],
                            )
                        nc.vector.tensor_add(su[:], su[:], ub[:])
                        iv = sup.tile([128, R * D], f32, tag="iv")
                        nc.sync.dma_start(iv[:], inv_d[m0 : m0 + 128, :])
                        nc.vector.tensor_mul(su[:], su[:], iv[:])
                        pt0 = ptp.tile([128, 128], f32, tag="pt")
                        nc.tensor.transpose(pt0[:], su[:, 0:128], eye[:])
                        nc.scalar.copy(t0[:, j * 128 : (j + 1) * 128], pt0[:])
                        pt1 = ptp.tile([128, 128], f32, tag="pt")
                        nc.tensor.transpose(pt1[:], su[:, 128:256], eye[:])
                        nc.scalar.copy(t1[:, j * 128 : (j + 1) * 128], pt1[:])
                    rh = cwp.tile([D, MCH], f32, tag="rh")
                    nc.sync.dma_start(rh[:, :Wc], rhs2T_d[:, ch0 : ch0 + Wc])
                    if same_rp:
                        pv = rh
                    else:
                        pv = cwp.tile([D, MCH], f32, tag="pv")
                        nc.sync.dma_start(pv[:, :Wc], prevT_d[:, ch0 : ch0 + Wc])
                    ph = php.tile([D, MCH], f32, tag="ph")
                    nc.tensor.matmul(
                        ph[:, :Wc], wt[:, 0:D], t0[:, :Wc], start=True, stop=False
                    )
                    nc.tensor.matmul(
                        ph[:, :Wc], wt[:, D : 2 * D], t1[:, :Wc], start=False, stop=False
                    )
                    nc.tensor.matmul(
                        ph[:, :Wc], wst[:], rh[:, :Wc], start=False, stop=True
                    )
                    hTt = cwp.tile([D, MCH], f32, tag="hT")
                    hT = hTt[:, :Wc]
                    oTt = cwp.tile([D, MCH], f32, tag="oT")
                    oT = oTt[:, :Wc]
                    nc.scalar.activation(hT, ph[:, :Wc], AF.Sigmoid, bias=cb[:])
                    if hT_out_d is not None:
                        nc.sync.dma_start(hT_out_d[:, ch0 : ch0 + Wc], hT)
                    ppr = pprp.tile([D, MCH], f32, tag="ppr")
                    nc.tensor.matmul(ppr[:, :Wc], pwth[:], hT, start=True, stop=False)
                    nc.tensor.matmul(
                        ppr[:, :Wc], pwtp[:], pv[:, :Wc], start=False, stop=True
                    )
                    pr = cwp.tile([D, MCH], f32, tag="pr")
                    nc.scalar.activation(pr[:, :Wc], ppr[:, :Wc], AF.Relu, bias=pb[:])
                    pg = pgp.tile([D, MCH], f32, tag="pg")
                    nc.tensor.matmul(pg[:, :Wc], twth[:], hT, start=True, stop=False)
                    nc.tensor.matmul(
                        pg[:, :Wc], twtp[:], pv[:, :Wc], start=False, stop=True
                    )
                    gt = cwp.tile([D, MCH], f32, tag="gt")
                    nc.scalar.activation(gt[:, :Wc], pg[:, :Wc], AF.Sigmoid, bias=tb[:])
                    # out = g*(pr-h) + h
                    tmp = cwp.tile([D, MCH], f32, tag="tmp")
                    nc.vector.tensor_sub(tmp[:, :Wc], pr[:, :Wc], hT)
                    nc.vector.tensor_mul(tmp[:, :Wc], tmp[:, :Wc], gt[:, :Wc])
                    nc.vector.tensor_add(oT, tmp[:, :Wc], hT)
                    if oT_out_d is not None:
                        nc.sync.dma_start(oT_out_d[:, ch0 : ch0 + Wc], oT)
                    for j in range(Wc // 128):
                        m0 = ch0 + j * 128
                        nvalid = min(128, NPC - m0)
                        if nvalid <= 0:
                            continue
                        po = pop.tile([128, D], f32, tag="po")
                        nc.tensor.transpose(
                            po[:], oTt[:, j * 128 : (j + 1) * 128], eye[0:D, 0:D]
                        )
                        rw = rowp.tile([128, D], f32, tag="rw")
                        nc.vector.tensor_copy(rw[:], po[:])
                        nc.sync.dma_start(
                            rows_dst[m0 : m0 + nvalid, :], rw[:nvalid, :]
                        )

            # ---- layer 1 ----
            stop = cfg.get("STOP", "full")
            if stop != "zero":
                edge_phase(1, x_d)
            if stop in ("dense1", "collective", "edges2", "full"):
                dense_phase(1, xT_d, xT_d, h1T_d, g1T_d, g1rows_d)
            if stop in ("collective", "edges2", "full"):
                from concourse import mybir as _mb
                nc.gpsimd.collective_compute(
                    "AllGather",
                    _mb.AluOpType.bypass,
                    replica_groups=[list(range(nC))],
                    ins=[g1rows_d.ap().opt()],
                    outs=[g1full_d.ap().opt()],
                )
            # ---- layer 2 ----
            if stop in ("edges2", "full"):
                edge_phase(2, g1full_d)
            if stop == "full":
                dense_phase(2, g1T_d, h1T_d, None, None, out_d)
            else:
                # keep "out" written so results exist
                dummy = rowp.tile([128, D], f32, tag="rw")
                nc.vector.memset(dummy[:], 0.0)
                for m0 in range(0, NPC, 128):
                    nv = min(128, NPC - m0)
                    nc.sync.dma_start(out_d[m0 : m0 + nv, :], dummy[:nv, :])

    nc.compile()
    return nc


def _host_arrays(cfg, x, weights):
    """Per-core input arrays shared across cores + per-core xT."""
    NPC, NPAD = cfg["NPC"], cfg["NPAD"]
    nC = cfg["C"]
    com = {}
    com["x"] = np.ascontiguousarray(x, dtype=np.float32)
    com["eye"] = np.eye(128, dtype=np.float32)
    for L in (1, 2):
        w = weights[f"conv{L}_w"]; b = weights[f"conv{L}_b"]
        ws = weights[f"conv{L}_ws"]; bs = weights[f"conv{L}_bs"]
        pw = weights[f"hw{L}_pw"]; pbv = weights[f"hw{L}_pb"]
        tw = weights[f"hw{L}_tw"]; tbv = weights[f"hw{L}_tb"]
        wT = np.ascontiguousarray(w.T, np.float32)              # [R*D, D]
        com[f"wt{L}"] = np.ascontiguousarray(
            np.concatenate([wT[0:128], wT[128:256]], axis=1), np.float32
        )
        com[f"wst{L}"] = np.ascontiguousarray(ws.T, np.float32)
        com[f"cb{L}"] = np.ascontiguousarray((b + bs)[:, None], np.float32)
        pwT = np.ascontiguousarray(pw.T, np.float32)            # [2D, D]
        com[f"pwth{L}"] = np.ascontiguousarray(pwT[0:D], np.float32)
        com[f"pwtp{L}"] = np.ascontiguousarray(pwT[D : 2 * D], np.float32)
        com[f"pb{L}"] = np.ascontiguousarray(pbv[:, None], np.float32)
        twT = np.ascontiguousarray(tw.T, np.float32)
        com[f"twth{L}"] = np.ascontiguousarray(twT[0:D], np.float32)
        com[f"twtp{L}"] = np.ascontiguousarray(twT[D : 2 * D], np.float32)
        com[f"tb{L}"] = np.ascontiguousarray(tbv[:, None], np.float32)
    xTs = []
    for c in range(nC):
        xT = np.zeros((D, NPAD), np.float32)
        xT[:, :NPC] = com["x"][c * NPC : (c + 1) * NPC].T
        xTs.append(xT)
    return com, xTs


def _inv_arrays(cfg, dst, rel):
    NPC, NPAD = cfg["NPC"], cfg["NPAD"]
    nC, nN = cfg["C"], cfg["N"]
    dst = np.asarray(dst).astype(np.int64)
    rel = np.asarray(rel).astype(np.int64)
    cnt = np.bincount(dst * R + rel, minlength=nN * R).reshape(nN, R)
    inv = (1.0 / np.maximum(cnt, 1)).astype(np.float32)  # [N, R]
    invs = []
    for c in range(nC):
        a = np.zeros((NPAD, R * D), np.float32)
        blk = inv[c * NPC : (c + 1) * NPC]                # [NPC, R]
        a[:NPC] = np.repeat(blk, D, axis=1)
        invs.append(a)
    return invs


def make_in_maps(cfg, inputs):
    layout, gblobs, sblobs = _prep(
        cfg, inputs["src"], inputs["dst"], inputs["rel"]
    )
    com, xTs = _host_arrays(cfg, inputs["x"], inputs)
    invs = _inv_arrays(cfg, inputs["dst"], inputs["rel"])
    in_maps = []
    for c in range(cfg["C"]):
        m = dict(com)
        m["xT"] = xTs[c]
        m["gidx"] = gblobs[c]
        m["sidx"] = sblobs[c]
        m["inv"] = invs[c]
        in_maps.append(m)
    return layout, in_maps


LAST_RESULT = None


def kernel(**inputs) -> np.ndarray:
    global LAST_RESULT
    from concourse.bass_utils import run_bass_kernel_spmd

    cfg = FULL_CFG
    layout, in_maps = make_in_maps(cfg, inputs)
    nc = _build(cfg, layout)
    res = run_bass_kernel_spmd(nc, in_maps, list(range(cfg["C"])))
    LAST_RESULT = res
    out = np.concatenate([res.results[c]["out"] for c in range(cfg["C"])], axis=0)
    return out.astype(np.float32)
